# revision 1
# baseline (speedup 1.0000x reference)
"""AdaptiveTokenMixer Trainium2 kernel (8 NeuronCores, pure data parallel).

Per-core algorithm (one batch element per core):
  1. alpha stage: delta_times/valid_mask loaded as per-block rows (one
     contiguous DMA each), sliding windows formed by PE transposes of
     free-dim-shifted slices; masked temporal-decay softmax over K=8 offsets
     blended with host-precomputed (b/(1-b))*softmax(w) (scale-invariant
     under the final renormalization), masked + renormalized -> alpha bf16.
  2. W stage: alpha is written to a DRAM scratch with a SKEWED access
     pattern (single multi-dim DMA), forming banded W^T[m, k] =
     alpha[n0+m, k-m] per 120-position block (m-major 128x128 tiles over a
     zeros-initialized input buffer); loaded back naturally (single DMA) and
     PE-transposed to W[k, m].
  3. Mix: out[m, :] = sum_k W[k, m] * x[n0+k, :] -- one 128x128 @ 128x256
     bf16 matmul per block realizes the K-tap mixing exactly (PSUM f32).
  4. Evict PSUM -> SBUF bf16 (DVE/ACT alternating), single batched DMA out.

Self-contained: hardcodes shapes for B=8, N=4096, d=256, K=8.
"""
import numpy as np
import ml_dtypes

import concourse.bass as bass
import concourse.bacc as bacc
import concourse.mybir as mybir
from concourse import tile
from concourse.bass_utils import run_bass_kernel_spmd

B, N, D, K = 8, 4096, 256, 8
BLK = 120                      # output positions per block
NB = (N + BLK - 1) // BLK      # 35 blocks -> covers 4200 positions
NOUT = NB * BLK                # 4200 rows in padded device output
NPAD = 4224                    # padded input length (>= 34*120 + 136)
KW = 128                       # k-window (contraction) per block
WBLK = KW * KW                 # W scratch elements per block
F = K * NB                     # alpha free size (b-major, p-minor)
BIG = 1024.0

_CACHE = {}


def _build():
    nc = bacc.Bacc("TRN2", target_bir_lowering=False, debug=False,
                   num_devices=B)
    f32 = mybir.dt.float32
    bf16 = mybir.dt.bfloat16

    x_t = nc.dram_tensor("x", [NPAD, D], bf16, kind="ExternalInput")
    dt_t = nc.dram_tensor("dt", [NPAD], f32, kind="ExternalInput")
    vf_t = nc.dram_tensor("vf", [NPAD], f32, kind="ExternalInput")
    bwsm_t = nc.dram_tensor("bwsm", [128, K], f32, kind="ExternalInput")
    idf_t = nc.dram_tensor("idf", [128, 128], f32, kind="ExternalInput")
    idb_t = nc.dram_tensor("idb", [128, 128], bf16, kind="ExternalInput")
    wz_t = nc.dram_tensor("wz", [NB * WBLK], bf16, kind="ExternalInput")
    out_t = nc.dram_tensor("out", [NOUT, D], bf16, kind="ExternalOutput")

    def pb(t):  # [128,(b,p)] view -> [128, b, p] (p innermost, for reduces)
        return bass.AP(t.tensor, t.offset, [t.ap[0], [K, NB], [1, K]])

    def strip(t, p):  # tap-p strip [128, NB] (stride K)
        return bass.AP(t.tensor, t.offset + p, [t.ap[0], [K, NB]])

    def exp_nb(a):  # [128, NB] AP -> [128, NB, (K-rep)]
        return bass.AP(a.tensor, a.offset, [a.ap[0], list(a.ap[1]), [0, K]])

    def exp_k(a):  # [128, K] AP -> [128, (NB-rep), K]
        return bass.AP(a.tensor, a.offset, [a.ap[0], [0, NB], [1, K]])

    with tile.TileContext(nc) as tc:
        with tc.tile_pool(name="alph", bufs=1) as apool, \
             tc.tile_pool(name="mix", bufs=4) as mpool, \
             tc.tile_pool(name="big", bufs=1) as bpool, \
             tc.tile_pool(name="psA", bufs=2, space="PSUM") as psA, \
             tc.tile_pool(name="psB", bufs=3, space="PSUM") as psB:

            # ---- constant / input loads (one DMA each) ----
            ident_f = apool.tile([128, 128], f32)
            nc.sync.dma_start(ident_f[:], idf_t.ap())
            ident_b = apool.tile([128, 128], bf16)
            nc.scalar.dma_start(ident_b[:], idb_t.ap())
            bwsm = apool.tile([128, K], f32)
            nc.sync.dma_start(bwsm[:], bwsm_t.ap())
            dt_rows = apool.tile([35, 136], f32)
            nc.sync.dma_start(dt_rows[:],
                              bass.AP(dt_t, 0, [[BLK, NB], [1, 136]]))
            vf_rows = apool.tile([35, 136], f32)
            nc.sync.dma_start(vf_rows[:],
                              bass.AP(vf_t, 0, [[BLK, NB], [1, 136]]))
            # all 35 x windows in one DMA: x_all[i, b, d] = x[b*120+i, d]
            x_all = bpool.tile([128, NB, D], bf16)
            nc.scalar.dma_start(
                x_all[:], bass.AP(x_t, 0, [[D, 128], [BLK * D, NB], [1, D]]))

            # ---- window strips via PE transpose ----
            dtw = apool.tile([128, F], f32)
            vw = apool.tile([128, F], f32)
            for p in range(K):
                for rows, dst in ((dt_rows, dtw), (vf_rows, vw)):
                    pt = psA.tile([128, NB], f32, tag="win")
                    nc.tensor.transpose(pt[:], rows[:NB, p:p + 128],
                                        ident_f[:NB, :NB])
                    nc.vector.tensor_copy(strip(dst, p), pt[:])

            # ---- alpha stage ----
            t1 = apool.tile([128, F], f32)
            nc.vector.tensor_scalar(t1[:], dtw[:], -1.0, BIG,
                                    mybir.AluOpType.mult, mybir.AluOpType.add)
            cv = apool.tile([128, F], f32)
            nc.vector.tensor_tensor(cv[:], vw[:], exp_nb(strip(vw, 0)),
                                    mybir.AluOpType.mult)
            lg = apool.tile([128, F], f32)
            nc.vector.tensor_tensor(lg[:], t1[:], cv[:], mybir.AluOpType.mult)
            mx = apool.tile([128, NB], f32)
            nc.vector.tensor_reduce(mx[:], pb(lg), mybir.AxisListType.X,
                                    mybir.AluOpType.max)
            ei = apool.tile([128, F], f32)
            nc.vector.tensor_tensor(ei[:], lg[:], exp_nb(mx[:, :]),
                                    mybir.AluOpType.subtract)
            e = apool.tile([128, F], f32)
            nc.scalar.activation(e[:], ei[:], mybir.ActivationFunctionType.Exp)
            s = apool.tile([128, NB], f32)
            nc.vector.tensor_reduce(s[:], pb(e), mybir.AxisListType.X,
                                    mybir.AluOpType.add)
            rcp = apool.tile([128, NB], f32)
            nc.vector.reciprocal(rcp[:], s[:])
            th = apool.tile([128, F], f32)
            nc.vector.tensor_tensor(th[:], e[:], exp_nb(rcp[:, :]),
                                    mybir.AluOpType.mult)
            au = apool.tile([128, F], f32)
            nc.vector.tensor_tensor(au[:], th[:], exp_k(bwsm[:, :]),
                                    mybir.AluOpType.add)
            nc.vector.tensor_tensor(au[:], au[:], cv[:], mybir.AluOpType.mult)
            sa = apool.tile([128, NB], f32)
            nc.vector.tensor_reduce(sa[:], pb(au), mybir.AxisListType.X,
                                    mybir.AluOpType.add)
            nc.vector.tensor_scalar(sa[:], sa[:], 1e-8, None,
                                    mybir.AluOpType.max)
            r = apool.tile([128, NB], f32)
            nc.vector.reciprocal(r[:], sa[:])
            nc.vector.tensor_tensor(r[:], r[:], strip(vw, 0),
                                    mybir.AluOpType.mult)
            af = apool.tile([128, F], bf16)
            nc.vector.tensor_tensor(af[:], au[:], exp_nb(r[:, :]),
                                    mybir.AluOpType.mult)

            # ---- skewed W write (single DMA): W^T[b][m, m+p] = af[m, p, b]
            nc.sync.dma_start(
                bass.AP(wz_t, 0, [[KW + 1, BLK], [WBLK, NB], [1, K]]),
                bass.AP(af.tensor, af.offset, [af.ap[0], [K, NB], [1, K]])[:BLK, :, :])

            # ---- W^T load back (single DMA, natural m-major) ----
            wT_all = bpool.tile([128, NB, KW], bf16)
            nc.scalar.dma_start(
                wT_all[:],
                bass.AP(wz_t, 0, [[KW, 128], [WBLK, NB], [1, KW]]))

            # ---- per-block: PE transpose W^T -> W; banded matmul; evict ----
            out_all = bpool.tile([128, NB, D], bf16)
            for b in range(NB):
                wtp = psA.tile([KW, KW], bf16, tag="wt")
                nc.tensor.transpose(wtp[:], wT_all[:, b, :], ident_b[:])
                wt = mpool.tile([KW, KW], bf16, tag="w")
                if b % 2 == 0:
                    nc.vector.tensor_copy(wt[:], wtp[:])
                else:
                    nc.scalar.copy(wt[:], wtp[:])
                pt = psB.tile([KW, D], f32, tag="mm")
                nc.tensor.matmul(pt[:], wt[:], x_all[:, b, :])
                if b % 2 == 0:
                    nc.scalar.copy(out_all[:BLK, b, :], pt[:BLK, :])
                else:
                    nc.vector.tensor_copy(out_all[:BLK, b, :], pt[:BLK, :])

            # ---- single batched store: out[b*120+i, d] = out_all[i, b, d]
            nc.sync.dma_start(
                bass.AP(out_t, 0, [[D, BLK], [BLK * D, NB], [1, D]]),
                out_all[:BLK, :, :])
    nc.compile()
    return nc


def _get_nc():
    if "nc" not in _CACHE:
        _CACHE["nc"] = _build()
    return _CACHE["nc"]


def _make_in_maps(x, delta_times, valid_mask, w, beta):
    w64 = w.astype(np.float64)
    wsm = np.exp(w64 - w64.max())
    wsm /= wsm.sum()
    b = 1.0 / (1.0 + np.exp(-float(beta[0])))
    bwsm = np.tile((b / (1.0 - b) * wsm)[None, :], (128, 1)).astype(np.float32)
    ident = np.eye(128, dtype=np.float32)
    wz = np.zeros(NB * WBLK, np.float32).astype(ml_dtypes.bfloat16)

    in_maps = []
    for i in range(B):
        xp = np.zeros((NPAD, D), np.float32)
        xp[:N] = x[i]
        dtp = np.zeros(NPAD, np.float32)
        dtp[:N] = delta_times[i]
        vfp = np.zeros(NPAD, np.float32)
        vfp[:N] = valid_mask[i].astype(np.float32)
        in_maps.append({
            "x": xp.astype(ml_dtypes.bfloat16),
            "dt": dtp,
            "vf": vfp,
            "bwsm": bwsm,
            "idf": ident,
            "idb": ident.astype(ml_dtypes.bfloat16),
            "wz": wz,
        })
    return in_maps


def _execute(in_maps, trace=False, **kw):
    nc = _get_nc()
    return run_bass_kernel_spmd(nc, in_maps, core_ids=list(range(B)),
                                trace=trace, **kw)


def kernel(x, delta_times, valid_mask, w, beta):
    in_maps = _make_in_maps(x, delta_times, valid_mask, w, beta)
    kr = _execute(in_maps, trace=False)
    outs = [kr.results[i]["out"][:N].astype(np.float32) for i in range(B)]
    return np.stack(outs, axis=0)

